# revision 44
# baseline (speedup 1.0000x reference)
"""Trainium2 Bass kernel for BaseSegHead (dynamic 1x1-conv seg logits).

Computes, for full inputs:
    qry_feats = in_feats @ qry_w.T + qry_b                  [1200, 32]
    key_map   = einsum('oc,bchw->bohw', key_w, feat_map) + key_b
    logits    = einsum('bnc,bchw->bnhw', qry_feats.reshape(4,300,32), key_map)
    out       = logits.reshape(1200, 160, 160)

Sharding: 8 cores = 4 batch images x 2 spatial (H) halves. Core c handles
batch b = c//2, rows h*80:(h+1)*80. Each core reads feat_map[b,:,rows,:],
its 300 queries, and writes a [300, 80*160] output shard -- no cross-core
communication and no duplicated feat_map reads.

Precision: matmul operands ship as fp16 (full PE rate; halves input DMA
bytes); accumulation stays fp32 in PSUM.  The OUTPUT is int8: 1/OUT_STEP
is folded into qry_w/qry_b on the host, so PSUM holds logits/OUT_STEP and
the drains quantize for free via the engines' native fp32->int8
round-to-nearest-even + saturation (verified on HW for ACT/DVE/Pool).
Host dequantizes (*OUT_STEP).  Measured error: absmax/scale 0.44%, l2
1.52% vs the 2% gate.  int8 feat_map INPUT as well measures 1.97% -- too
close to the gate; fp8 anywhere fails outright (~3-3.6%).

Layout (trace-driven, ~47-49us vs 53.1us for the fp16-output version,
60.6us original): traffic is 6.75 MB in + 3.84 MB out per core; the 16
DMA engines sustain ~26 GB/s each (~418 GB/s; per-byte rate is flat for
descriptors >=512 B, so 2-8 KB partition rows are all full-rate).  After
the int8 switch the kernel is BALANCED: input DMA ~17us, V/S drain
trains ~26us/engine, PE ~23.5us, so stall elimination, not byte count,
governs.  Input rides the SP sync HW-DGE ring (tail 512-col block first:
it primes the pipeline; then six 2048-col blocks, d0|d1 interleaved,
8 KB rows).  Out DMAs are triggered from the Pool engine via SWDGE
(~25ns seq + ~1us Pool-engine descriptor-gen each) -- a HWDGE trigger
costs ~1.2us of the issuing SEQUENCER and the SP seq was 93% busy, the
co-bottleneck of the fp16 version.  Out groups: tail first, then 4096-col
quarters with the last quarter split 2048+2048, each group's 3 chunk
triggers emitted (in Pool program order) right after the block whose
drains complete it.  PSUM drains are single-bank [*, 512] copies with
SEVEN main PSUM buffers, assigned to scalar/vector by accumulated-cost
balance with weights s=720/v=678 (copies) and s=686/v=678 (bias) -- the
BALANCE PATTERN, not the absolute ns, is what matters: re-weighting to
the measured slice times (s 576/ v 598) costs +6us.  The next block's
key-quad matmuls are split around j1 and its bias-add is emitted right
after quad half d1 (before j2's drains): in-order V/S queues otherwise
hold the bias behind 4 more drains while the PE idles ~2.3us at block
boundaries.  A dummy activation at t~0 preloads the scalar ACT table
(~1.3us ACT_TABLE_LOAD).

Known dead ends (measured on HW): PE warm-up matmuls (HAM never
un-throttles; all matmuls run at the 1.2 GHz cold rate), 2-bank paired
drains (depth 7->3 singles->pairs: +2.6us; j2-only pairs at depth 1:
+14us -- PSUM depth beats instruction count), emitting the whole next
quad + bias after j0 (8-MM burst stalls PE: +5.3us), ct/bt consts on the
ACT HWDGE queue (+2.3us), per-chunk tail triggers inside the j-loop
(+0.7us), 1024-col final-group split (neutral/worse), per-512-col final
out DMAs (+4.8us).  GPSIMD cannot access PSUM (no Pool drains); only
SP/ACT/Pool can trigger DMAs.  Run-to-run HW variance is ~ +/-0.9us.

TensorE array tiling: the key projection (M=32) runs 4-way column-tiled
into one PSUM bank per quad of hw-tiles; one bias-activation drains four
tiles. The main einsum (K=32) runs 4-way row-tiled: hw-tile t keeps its
q and key_map operands on SBUF partitions 32*(t%4), so consecutive tiles
issue to distinct PE row-groups and overlap on the array.
"""

import os
import sys

sys.path.insert(0, "/opt/trn_rl_repo")
os.environ.setdefault("MYCRO_LOCAL_CACHE", "1")

import numpy as np

BATCH = 4
N_PER = 300
IN_DIM = 256
KEY_DIM = 32
FH = FW = 160
HHALF = FH // 2            # 80 rows per core
HW = HHALF * FW            # 12800 spatial positions per core
N_CORES = 8

MMN = 512                  # matmul moving free size (one fp32 PSUM bank)
N_T = HW // MMN            # 25 hw-tiles
N_BLK = 6                  # six full 2048-col blocks (quads) + one 512 tail
BLKW = 4 * MMN             # 2048 feat columns per block
# out-DMA groups: tail tile first (it drains first), then 4096-col
# quarters (4KB int8 rows), with the final quarter split in two so the
# last DMA after the last drain is ~2us shorter.

N_CHUNKS = ((0, 128), (128, 128), (256, 44))   # query-row chunks (300 rows)
CPACK_W = 728              # fp16: qry_wT (64) + in_featsT (600) + key_wT (64)
FPW = 2 * HW               # featP width: d0|d1 interleaved per block

# int8 output quantization: logits are computed as logits/OUT_STEP on device
# (1/OUT_STEP folded into qry_w/qry_b on host), drained to int8 with the
# engines' native round-to-nearest-even + saturation, and rescaled on host.
# max |logit| = 24.24 -> max |q| = 125.7 < 127: no saturation. Measured
# error: absmax/scale 0.40%, l2 1.52% (gate 2%).
OUT_STEP = 24.5 / 127.0

_CACHE = {}


def build_nc():
    import concourse.bass as bass
    import concourse.bacc as bacc
    import concourse.mybir as mybir
    from concourse import tile

    f32 = mybir.dt.float32
    f16 = mybir.dt.float16
    i8 = mybir.dt.int8
    Ident = mybir.ActivationFunctionType.Identity

    nc = bacc.Bacc("TRN2", target_bir_lowering=False, debug=False)

    featP = nc.dram_tensor("featP", [128, FPW], f16, kind="ExternalInput")
    cpack = nc.dram_tensor("cpack", [128, CPACK_W], f16, kind="ExternalInput")
    bpack = nc.dram_tensor("bpack", [128, 2], f32, kind="ExternalInput")
    out = nc.dram_tensor("out", [128, 3 * HW], i8, kind="ExternalOutput")

    with tile.TileContext(nc) as tc:
        with (
            tc.tile_pool(name="const", bufs=1) as cpool,
            tc.tile_pool(name="fpool", bufs=1) as fpool,
            tc.tile_pool(name="opool", bufs=1) as opool,
            tc.tile_pool(name="kmap", bufs=1) as kpool,
            tc.tile_pool(name="ps_main", bufs=7, space=bass.MemorySpace.PSUM) as ps_main,
            tc.tile_pool(name="ps_small", bufs=1, space=bass.MemorySpace.PSUM) as ps_small,
        ):
            # --- DMA ring head: consts, then the 7 paired feat blocks ----
            # ct/bt ride the ACT sequencer's HWDGE queue: their completion
            # semaphores are then independent of the SP input firehose
            # (sharing its queues delays the completion update ~2us behind
            # the queued feat blocks, stalling the qry projection), and the
            # SP queue's first trigger becomes the priming tail block.
            ct = cpool.tile([128, CPACK_W], f16, name="ct")
            nc.sync.dma_start(ct[:], cpack[:])
            bt = cpool.tile([128, 2], f32, name="bt")
            nc.sync.dma_start(bt[:], bpack[:])
            qw = (ct[:, 0:32], ct[:, 32:64])
            inT = (ct[:, 64:364], ct[:, 364:664])
            kw = (ct[:, 664:696], ct[:, 696:728])
            qb = bt[:, 0:1]        # qry_b replicated in all four bands
            kb = bt[:, 1:2]        # key_b replicated in all four bands

            # featP block k: cols [2k*BLKW, (2k+2)*BLKW) = d0 block | d1 block
            # The tiny tail block loads FIRST: it lands ~1us before block 0
            # and primes the whole matmul->drain->out pipeline.
            fp = fpool.tile([128, FPW], f16, name="fp")
            nc.sync.dma_start(fp[:, 2 * N_BLK * BLKW:FPW],
                              featP[:, 2 * N_BLK * BLKW:FPW])
            for k in range(N_BLK):
                nc.sync.dma_start(
                    fp[:, 2 * k * BLKW:2 * (k + 1) * BLKW],
                    featP[:, 2 * k * BLKW:2 * (k + 1) * BLKW],
                )

            def feat(d, t):
                # hw-tile t, channel half d -> fp column range
                k = t // 4
                if k < N_BLK:
                    c0 = 2 * k * BLKW + d * BLKW + (t % 4) * MMN
                else:
                    c0 = 2 * N_BLK * BLKW + d * MMN
                return fp[:, c0:c0 + MMN]

            # Preload the scalar-engine activation table with a dummy op at
            # t~0 (vector memsets a scratch tile first) so the ~1.3us
            # ACT_TABLE_LOAD doesn't delay the first real activation.
            warm = cpool.tile([128, 8], f32, name="warm")
            nc.vector.memset(warm[:], 0.0)
            warm16 = cpool.tile([128, 8], f16, name="warm16")
            nc.scalar.activation(warm16[:], warm[:, 0:8], Ident, bias=warm[:, 0:1])

            # --- qry projection, 4-way column-tiled (4 band copies) -------
            # Band 0 is projected and activated FIRST: the tail hw-tile
            # (t=24, band 0) and its 3 mains gate the whole ramp, and this
            # unblocks them ~1us before the full q_sb is ready.
            qp = ps_small.tile([128, MMN], f32, name="qp", tag="kp")
            q_sb = cpool.tile([128, N_PER], f16, name="q_sb")
            for b in range(4):
                for d in range(2):
                    nc.tensor.matmul(
                        qp[32 * b:32 * b + 32, 0:N_PER],
                        qw[d],
                        inT[d],
                        start=(d == 0),
                        stop=(d == 1),
                        tile_position=(0, 32 * b),
                    )
                # PSUM APs not starting at partition 0 are limited to one
                # 32-partition group, so activate per band.
                p = 32 * b
                nc.scalar.activation(q_sb[p:p + 32, :], qp[p:p + 32, 0:N_PER],
                                     Ident, bias=qb[p:p + 32, :])

            # --- key_map: 4-way column-tiled, banded layout ---------------
            # hw-tile t lives on SBUF partitions 32*(t%4), columns
            # (t//4)*512; one [128,512] PSUM bank holds a whole quad and is
            # drained by a single bias-activation.
            key_map = kpool.tile([128, 7 * MMN], f16, name="key_map")

            # Drains and key-quad bias-adds are assigned to scalar/vector by
            # accumulated-cost balance (GPSIMD cannot access PSUM, so Pool
            # only runs the out-DMA SWDGE triggers).
            acc = {"s": 0.0, "v": 0.0, "g": 0.0}

            def drain(dst, src, cs=720, cv=678):
                if acc["s"] + cs < acc["v"] + cv:
                    nc.scalar.copy(dst, src)
                    acc["s"] += cs
                else:
                    nc.vector.tensor_copy(dst, src)
                    acc["v"] += cv

            def bias_add(dst, src, bias):
                if acc["s"] + 686 < acc["v"] + 678:
                    nc.scalar.activation(dst, src, Ident, bias=bias)
                    acc["s"] += 686
                else:
                    nc.vector.tensor_scalar_add(dst, src, bias)
                    acc["v"] += 678

            KP = {}

            def quad_mms(k, d):
                # d outer, band inner: each round's four column-group
                # matmuls overlap on the PE array.
                if d == 0:
                    KP[k] = ps_small.tile([128, MMN], f32, name=f"kp_{k}", tag="kp")
                nb = min(4, N_T - 4 * k)
                for b in range(nb):
                    nc.tensor.matmul(
                        KP[k][32 * b:32 * b + 32, :],
                        kw[d],
                        feat(d, 4 * k + b),
                        start=(d == 0),
                        stop=(d == 1),
                        tile_position=(0, 32 * b),
                    )

            def quad_bias(k):
                nb = min(4, N_T - 4 * k)
                p = 32 * nb
                bias_add(key_map[0:p, k * MMN:(k + 1) * MMN], KP[k][0:p, :],
                         kb[0:p, :])

            def key_quad(k):
                quad_mms(k, 0)
                quad_mms(k, 1)
                quad_bias(k)

            # --- output row-buffers: one [*, 12800] int8 tile per chunk ---
            OB = [opool.tile([128, HW], i8, name=f"ob_{j}") for j in range(3)]

            # --- main einsum: 4-way row-tiled over band b = t%4 -----------
            # chunk-outer / tile-inner order: adjacent matmuls target
            # different PE row-groups and overlap on the array.  (Pairing
            # even/odd tiles into two-bank PSUM tiles with one [m, 1024]
            # drain measures 2.6us SLOWER: pipeline depth drops 7 -> 3
            # and the PE stalls on PSUM-free.)
            def main_tiles(j, tiles):
                n0, m = N_CHUNKS[j]
                for t in tiles:
                    b = t % 4
                    kcol = (t // 4) * MMN
                    mp = ps_main.tile([128, MMN], f32, name=f"mp_{t}_{n0}", tag="mp")
                    nc.tensor.matmul(
                        mp[:m, :],
                        q_sb[32 * b:32 * b + 32, n0:n0 + m],
                        key_map[32 * b:32 * b + 32, kcol:kcol + MMN],
                        tile_position=(32 * b, 0),
                    )
                    drain(OB[j][:m, t * MMN:(t + 1) * MMN], mp[:m, :])

            # out DMAs are issued from the Pool engine (SWDGE): ~25ns of
            # sequencer time per trigger vs ~1.2us of HWDGE config that
            # would serialize on the Sync sequencer (which handles the
            # input stream).  Pool is an in-order engine that also runs
            # drains now, so each group's triggers are interleaved into
            # the emission stream right after the block that completes
            # the group.
            def emit_out_j(j, c0, c1):
                m = N_CHUNKS[j][1]
                acc["g"] += 1040
                nc.gpsimd.dma_start(
                    out[0:m, j * HW + c0:j * HW + c1], OB[j][0:m, c0:c1]
                )

            def emit_out(c0, c1):
                for j in range(3):
                    emit_out_j(j, c0, c1)

            # Interleave: tail quad/tile first (its data lands first), then
            # each key quad feeds its four hw-tiles.  Quads are emitted one
            # block AHEAD of their main tiles so the tensor stream never
            # stalls waiting for the current block's bias-add: while
            # bias_add(k) pends, the PE runs quad k+1's matmuls; the next
            # quad's matmuls are spread between the current block's
            # chunk-mains (4-MM half-rounds, not one 8-MM burst).
            key_quad(6)
            for j in range(3):
                main_tiles(j, (24,))
            quad_mms(0, 0)
            quad_mms(0, 1)
            quad_bias(0)
            for k in range(6):
                nxt = k + 1 if k < 5 else None
                for j in range(3):
                    main_tiles(j, range(4 * k, 4 * k + 4))
                    if nxt is not None and j < 2:
                        quad_mms(nxt, j)
                        # bias emitted before j2's drains: the in-order V/S
                        # queue would otherwise hold it behind 4 more drains
                        # while the PE idles at the block boundary.  (Moving
                        # the whole quad + bias to j0 measures 5.3us SLOWER:
                        # the 8-MM burst stalls the PE.)
                        if j == 1:
                            quad_bias(nxt)
                if k == 0:
                    emit_out(24 * 512, HW)          # tail group
                elif k == 1:
                    emit_out(0, 4096)
                elif k == 3:
                    emit_out(4096, 8192)
                elif k == 4:
                    emit_out(8192, 10240)
            emit_out(10240, 12288)

    nc.compile()
    return nc


def _get_nc():
    if "nc" not in _CACHE:
        _CACHE["nc"] = build_nc()
    return _CACHE["nc"]


def make_in_maps(in_feats, feat_map, qry_w, qry_b, key_b, key_w):
    # 1/OUT_STEP folded into the qry projection: PSUM then holds
    # logits/OUT_STEP, so the int8 drain is a pure (rounding) copy.
    qwT = (qry_w.T / OUT_STEP).astype(np.float16)             # [256, 32]
    kwT = key_w.T.astype(np.float16)                          # [256, 32]
    bpack = np.zeros((128, 2), np.float32)
    bpack[:, 0] = np.tile(qry_b / OUT_STEP, 4)
    bpack[:, 1] = np.tile(key_b, 4)
    in_maps = []
    for c in range(N_CORES):
        b, h = divmod(c, 2)
        ifT = in_feats[b * N_PER:(b + 1) * N_PER].T.astype(np.float16)
        cpack = np.zeros((128, CPACK_W), np.float16)
        cpack[:, 0:32] = qwT[0:128]
        cpack[:, 32:64] = qwT[128:256]
        cpack[:, 64:364] = ifT[0:128]
        cpack[:, 364:664] = ifT[128:256]
        cpack[:, 664:696] = kwT[0:128]
        cpack[:, 696:728] = kwT[128:256]
        feat16 = np.ascontiguousarray(
            feat_map[b, :, h * HHALF:(h + 1) * HHALF, :]
        ).reshape(IN_DIM, HW).astype(np.float16)
        # featP: block k holds cols [2k*BLKW,(2k+2)*BLKW) = d0 cols | d1 cols
        featP = np.empty((128, FPW), np.float16)
        for k in range(N_BLK + 1):
            w = BLKW if k < N_BLK else MMN
            c0 = k * BLKW
            for d in range(2):
                featP[:, 2 * c0 + d * w:2 * c0 + (d + 1) * w] = (
                    feat16[d * 128:(d + 1) * 128, c0:c0 + w]
                )
        in_maps.append({
            "featP": featP,
            "cpack": cpack,
            "bpack": bpack,
        })
    return in_maps


def kernel(**inputs):
    in_feats = np.asarray(inputs["in_feats"], dtype=np.float32)
    feat_map = np.asarray(inputs["feat_map"], dtype=np.float32)
    qry_w = np.asarray(inputs["qry_w"], dtype=np.float32)
    qry_b = np.asarray(inputs["qry_b"], dtype=np.float32)
    key_w = np.asarray(inputs["key_w"], dtype=np.float32)
    key_b = np.asarray(inputs["key_b"], dtype=np.float32)

    from concourse import bass_utils

    nc = _get_nc()
    in_maps = make_in_maps(in_feats, feat_map, qry_w, qry_b, key_b, key_w)
    trace = os.environ.get("SEG_KERNEL_TRACE", "0") == "1"
    res = bass_utils.run_bass_kernel_spmd(
        nc, in_maps, core_ids=list(range(N_CORES)), trace=trace
    )
    _CACHE["last_result"] = res

    out = np.empty((BATCH * N_PER, FH, FW), dtype=np.float32)
    for c in range(N_CORES):
        b, h = divmod(c, 2)
        raw = res.results[c]["out"].astype(np.float32) * OUT_STEP  # [128, 3*HW]
        shard = np.empty((N_PER, HW), dtype=np.float32)
        for j, (n0, m) in enumerate(N_CHUNKS):
            shard[n0:n0 + m] = raw[0:m, j * HW:(j + 1) * HW]
        out[b * N_PER:(b + 1) * N_PER, h * HHALF:(h + 1) * HHALF, :] = (
            shard.reshape(N_PER, HHALF, FW)
        )
    return out



# revision 46
# speedup vs baseline: 1.0075x; 1.0075x over previous
"""Trainium2 Bass kernel for BaseSegHead (dynamic 1x1-conv seg logits).

Computes, for full inputs:
    qry_feats = in_feats @ qry_w.T + qry_b                  [1200, 32]
    key_map   = einsum('oc,bchw->bohw', key_w, feat_map) + key_b
    logits    = einsum('bnc,bchw->bnhw', qry_feats.reshape(4,300,32), key_map)
    out       = logits.reshape(1200, 160, 160)

Sharding: 8 cores = 4 batch images x 2 spatial (H) halves. Core c handles
batch b = c//2, rows h*80:(h+1)*80. Each core reads feat_map[b,:,rows,:],
its 300 queries, and writes a [300, 80*160] output shard -- no cross-core
communication and no duplicated feat_map reads.

Precision: matmul operands ship as fp16 (full PE rate; halves input DMA
bytes); accumulation stays fp32 in PSUM.  The OUTPUT is int8: 1/OUT_STEP
is folded into qry_w/qry_b on the host, so PSUM holds logits/OUT_STEP and
the drains quantize for free via the engines' native fp32->int8
round-to-nearest-even + saturation (verified on HW for ACT/DVE/Pool).
Host dequantizes (*OUT_STEP).  Measured error: absmax/scale 0.44%, l2
1.52% vs the 2% gate.  int8 feat_map INPUT as well measures 1.97% -- too
close to the gate; fp8 anywhere fails outright (~3-3.6%).

Layout (trace-driven, ~47-49us vs 53.1us for the fp16-output version,
60.6us original): traffic is 6.75 MB in + 3.84 MB out per core; the 16
DMA engines sustain ~26 GB/s each (~418 GB/s; per-byte rate is flat for
descriptors >=512 B, so 2-8 KB partition rows are all full-rate).  After
the int8 switch the kernel is BALANCED: input DMA ~17us, V/S drain
trains ~26us/engine, PE ~23.5us, so stall elimination, not byte count,
governs.  Input rides the SP sync HW-DGE ring (tail 512-col block first:
it primes the pipeline; then six 2048-col blocks, d0|d1 interleaved,
8 KB rows).  Out DMAs are triggered from the Pool engine via SWDGE
(~25ns seq + ~1us Pool-engine descriptor-gen each) -- a HWDGE trigger
costs ~1.2us of the issuing SEQUENCER and the SP seq was 93% busy, the
co-bottleneck of the fp16 version.  Out groups: tail first, then 4096-col
quarters with the last quarter split 2048+2048, each group's 3 chunk
triggers emitted (in Pool program order) right after the block whose
drains complete it.  PSUM drains are single-bank [*, 512] copies with
SEVEN main PSUM buffers, assigned to scalar/vector by accumulated-cost
balance with weights s=720/v=678 (copies) and s=686/v=678 (bias) -- the
BALANCE PATTERN, not the absolute ns, is what matters: re-weighting to
the measured slice times (s 576/ v 598) costs +6us.  The next block's
key-quad matmuls are split around j1 and its bias-add is emitted right
after quad half d1 (before j2's drains): in-order V/S queues otherwise
hold the bias behind 4 more drains while the PE idles ~2.3us at block
boundaries.  A dummy activation at t~0 preloads the scalar ACT table
(~1.3us ACT_TABLE_LOAD).

Known dead ends (measured on HW): PE warm-up matmuls (HAM never
un-throttles; all matmuls run at the 1.2 GHz cold rate), 2-bank paired
drains (depth 7->3 singles->pairs: +2.6us; j2-only pairs at depth 1:
+14us -- PSUM depth beats instruction count), emitting the whole next
quad + bias after j0 (8-MM burst stalls PE: +5.3us), ct/bt consts on the
ACT HWDGE queue (+2.3us), per-chunk tail triggers inside the j-loop
(+0.7us), 1024-col final-group split (neutral/worse), per-512-col final
out DMAs (+4.8us).  GPSIMD cannot access PSUM (no Pool drains); only
SP/ACT/Pool can trigger DMAs.  Run-to-run HW variance is ~ +/-0.9us.

TensorE array tiling: the key projection (M=32) runs 4-way column-tiled
into one PSUM bank per quad of hw-tiles; one bias-activation drains four
tiles. The main einsum (K=32) runs 4-way row-tiled: hw-tile t keeps its
q and key_map operands on SBUF partitions 32*(t%4), so consecutive tiles
issue to distinct PE row-groups and overlap on the array.
"""

import os
import sys

sys.path.insert(0, "/opt/trn_rl_repo")
os.environ.setdefault("MYCRO_LOCAL_CACHE", "1")

import numpy as np

BATCH = 4
N_PER = 300
IN_DIM = 256
KEY_DIM = 32
FH = FW = 160
HHALF = FH // 2            # 80 rows per core
HW = HHALF * FW            # 12800 spatial positions per core
N_CORES = 8

MMN = 512                  # matmul moving free size (one fp32 PSUM bank)
N_T = HW // MMN            # 25 hw-tiles
N_BLK = 6                  # six full 2048-col blocks (quads) + one 512 tail
BLKW = 4 * MMN             # 2048 feat columns per block
# out-DMA groups: tail tile first (it drains first), then 4096-col
# quarters (4KB int8 rows), with the final quarter split in two so the
# last DMA after the last drain is ~2us shorter.

N_CHUNKS = ((0, 128), (128, 128), (256, 44))   # query-row chunks (300 rows)
CPACK_W = 728              # fp16: qry_wT (64) + in_featsT (600) + key_wT (64)
FPW = 2 * HW               # featP width: d0|d1 interleaved per block

# int8 output quantization: logits are computed as logits/OUT_STEP on device
# (1/OUT_STEP folded into qry_w/qry_b on host), drained to int8 with the
# engines' native round-to-nearest-even + saturation, and rescaled on host.
# max |logit| = 24.24 -> max |q| = 125.7 < 127: no saturation. Measured
# error: absmax/scale 0.40%, l2 1.52% (gate 2%).
OUT_STEP = 24.5 / 127.0

_CACHE = {}


def build_nc():
    import concourse.bass as bass
    import concourse.bacc as bacc
    import concourse.mybir as mybir
    from concourse import tile

    f32 = mybir.dt.float32
    f16 = mybir.dt.float16
    i8 = mybir.dt.int8
    Ident = mybir.ActivationFunctionType.Identity

    nc = bacc.Bacc("TRN2", target_bir_lowering=False, debug=False)

    featP = nc.dram_tensor("featP", [128, FPW], f16, kind="ExternalInput")
    cpack = nc.dram_tensor("cpack", [128, CPACK_W], f16, kind="ExternalInput")
    bpack = nc.dram_tensor("bpack", [128, 2], f32, kind="ExternalInput")
    out = nc.dram_tensor("out", [128, 3 * HW], i8, kind="ExternalOutput")

    with tile.TileContext(nc) as tc:
        with (
            tc.tile_pool(name="const", bufs=1) as cpool,
            tc.tile_pool(name="fpool", bufs=1) as fpool,
            tc.tile_pool(name="opool", bufs=1) as opool,
            tc.tile_pool(name="kmap", bufs=1) as kpool,
            tc.tile_pool(name="ps_main", bufs=7, space=bass.MemorySpace.PSUM) as ps_main,
            tc.tile_pool(name="ps_small", bufs=1, space=bass.MemorySpace.PSUM) as ps_small,
        ):
            # --- DMA ring head: consts, then the 7 paired feat blocks ----
            # ct/bt ride the ACT sequencer's HWDGE queue: their completion
            # semaphores are then independent of the SP input firehose
            # (sharing its queues delays the completion update ~2us behind
            # the queued feat blocks, stalling the qry projection), and the
            # SP queue's first trigger becomes the priming tail block.
            ct = cpool.tile([128, CPACK_W], f16, name="ct")
            nc.sync.dma_start(ct[:], cpack[:])
            bt = cpool.tile([128, 2], f32, name="bt")
            nc.sync.dma_start(bt[:], bpack[:])
            qw = (ct[:, 0:32], ct[:, 32:64])
            inT = (ct[:, 64:364], ct[:, 364:664])
            kw = (ct[:, 664:696], ct[:, 696:728])
            qb = bt[:, 0:1]        # qry_b replicated in all four bands
            kb = bt[:, 1:2]        # key_b replicated in all four bands

            # featP block k: cols [2k*BLKW, (2k+2)*BLKW) = d0 block | d1 block
            # The tiny tail block loads FIRST: it lands ~1us before block 0
            # and primes the whole matmul->drain->out pipeline.
            fp = fpool.tile([128, FPW], f16, name="fp")
            nc.sync.dma_start(fp[:, 2 * N_BLK * BLKW:FPW],
                              featP[:, 2 * N_BLK * BLKW:FPW])
            for k in range(N_BLK):
                if k < 2:
                    # ramp-critical blocks: d0|d1 halves as separate DMAs
                    # so quad k's d0 matmuls fire on the d0 semaphore
                    # ~1.2us before the full block lands.
                    for d in range(2):
                        c0 = (2 * k + d) * BLKW
                        nc.sync.dma_start(fp[:, c0:c0 + BLKW],
                                          featP[:, c0:c0 + BLKW])
                else:
                    nc.sync.dma_start(
                        fp[:, 2 * k * BLKW:2 * (k + 1) * BLKW],
                        featP[:, 2 * k * BLKW:2 * (k + 1) * BLKW],
                    )

            def feat(d, t):
                # hw-tile t, channel half d -> fp column range
                k = t // 4
                if k < N_BLK:
                    c0 = 2 * k * BLKW + d * BLKW + (t % 4) * MMN
                else:
                    c0 = 2 * N_BLK * BLKW + d * MMN
                return fp[:, c0:c0 + MMN]

            # Preload the scalar-engine activation table with a dummy op at
            # t~0 (vector memsets a scratch tile first) so the ~1.3us
            # ACT_TABLE_LOAD doesn't delay the first real activation.
            warm = cpool.tile([128, 8], f32, name="warm")
            nc.vector.memset(warm[:], 0.0)
            warm16 = cpool.tile([128, 8], f16, name="warm16")
            nc.scalar.activation(warm16[:], warm[:, 0:8], Ident, bias=warm[:, 0:1])

            # --- qry projection, 4-way column-tiled (4 band copies) -------
            # (Engine cost is COLUMN-driven: splitting the one [128, 300]
            # activation into per-band [32, 300] pieces quadruples its cost
            # and measures +2us.  PSUM APs starting at partition != 0 are
            # also limited to one 32-partition group.)
            qp = ps_small.tile([128, MMN], f32, name="qp", tag="kp")
            for b in range(4):
                for d in range(2):
                    nc.tensor.matmul(
                        qp[32 * b:32 * b + 32, 0:N_PER],
                        qw[d],
                        inT[d],
                        start=(d == 0),
                        stop=(d == 1),
                        tile_position=(0, 32 * b),
                    )
            q_sb = cpool.tile([128, N_PER], f16, name="q_sb")
            nc.scalar.activation(q_sb[:], qp[:, 0:N_PER], Ident, bias=qb)

            # --- key_map: 4-way column-tiled, banded layout ---------------
            # hw-tile t lives on SBUF partitions 32*(t%4), columns
            # (t//4)*512; one [128,512] PSUM bank holds a whole quad and is
            # drained by a single bias-activation.
            key_map = kpool.tile([128, 7 * MMN], f16, name="key_map")

            # Drains and key-quad bias-adds are assigned to scalar/vector by
            # accumulated-cost balance (GPSIMD cannot access PSUM, so Pool
            # only runs the out-DMA SWDGE triggers).
            acc = {"s": 0.0, "v": 0.0, "g": 0.0}

            def drain(dst, src, cs=720, cv=678):
                if acc["s"] + cs < acc["v"] + cv:
                    nc.scalar.copy(dst, src)
                    acc["s"] += cs
                else:
                    nc.vector.tensor_copy(dst, src)
                    acc["v"] += cv

            def bias_add(dst, src, bias):
                if acc["s"] + 686 < acc["v"] + 678:
                    nc.scalar.activation(dst, src, Ident, bias=bias)
                    acc["s"] += 686
                else:
                    nc.vector.tensor_scalar_add(dst, src, bias)
                    acc["v"] += 678

            KP = {}

            def quad_mms(k, d):
                # d outer, band inner: each round's four column-group
                # matmuls overlap on the PE array.
                if d == 0:
                    KP[k] = ps_small.tile([128, MMN], f32, name=f"kp_{k}", tag="kp")
                nb = min(4, N_T - 4 * k)
                for b in range(nb):
                    nc.tensor.matmul(
                        KP[k][32 * b:32 * b + 32, :],
                        kw[d],
                        feat(d, 4 * k + b),
                        start=(d == 0),
                        stop=(d == 1),
                        tile_position=(0, 32 * b),
                    )

            def quad_bias(k):
                nb = min(4, N_T - 4 * k)
                p = 32 * nb
                bias_add(key_map[0:p, k * MMN:(k + 1) * MMN], KP[k][0:p, :],
                         kb[0:p, :])

            def key_quad(k):
                quad_mms(k, 0)
                quad_mms(k, 1)
                quad_bias(k)

            # --- output row-buffers: one [*, 12800] int8 tile per chunk ---
            OB = [opool.tile([128, HW], i8, name=f"ob_{j}") for j in range(3)]

            # --- main einsum: 4-way row-tiled over band b = t%4 -----------
            # chunk-outer / tile-inner order: adjacent matmuls target
            # different PE row-groups and overlap on the array.  (Pairing
            # even/odd tiles into two-bank PSUM tiles with one [m, 1024]
            # drain measures 2.6us SLOWER: pipeline depth drops 7 -> 3
            # and the PE stalls on PSUM-free.)
            def main_tiles(j, tiles):
                n0, m = N_CHUNKS[j]
                for t in tiles:
                    b = t % 4
                    kcol = (t // 4) * MMN
                    mp = ps_main.tile([128, MMN], f32, name=f"mp_{t}_{n0}", tag="mp")
                    nc.tensor.matmul(
                        mp[:m, :],
                        q_sb[32 * b:32 * b + 32, n0:n0 + m],
                        key_map[32 * b:32 * b + 32, kcol:kcol + MMN],
                        tile_position=(32 * b, 0),
                    )
                    drain(OB[j][:m, t * MMN:(t + 1) * MMN], mp[:m, :])

            # out DMAs are issued from the Pool engine (SWDGE): ~25ns of
            # sequencer time per trigger vs ~1.2us of HWDGE config that
            # would serialize on the Sync sequencer (which handles the
            # input stream).  Pool is an in-order engine that also runs
            # drains now, so each group's triggers are interleaved into
            # the emission stream right after the block that completes
            # the group.
            def emit_out_j(j, c0, c1):
                m = N_CHUNKS[j][1]
                acc["g"] += 1040
                nc.gpsimd.dma_start(
                    out[0:m, j * HW + c0:j * HW + c1], OB[j][0:m, c0:c1]
                )

            def emit_out(c0, c1):
                for j in range(3):
                    emit_out_j(j, c0, c1)

            # Interleave: tail quad/tile first (its data lands first), then
            # each key quad feeds its four hw-tiles.  Quads are emitted one
            # block AHEAD of their main tiles so the tensor stream never
            # stalls waiting for the current block's bias-add: while
            # bias_add(k) pends, the PE runs quad k+1's matmuls; the next
            # quad's matmuls are spread between the current block's
            # chunk-mains (4-MM half-rounds, not one 8-MM burst).
            key_quad(6)
            for j in range(3):
                main_tiles(j, (24,))
            quad_mms(0, 0)
            quad_mms(0, 1)
            quad_bias(0)
            for k in range(6):
                nxt = k + 1 if k < 5 else None
                for j in range(3):
                    main_tiles(j, range(4 * k, 4 * k + 4))
                    if nxt is not None and j < 2:
                        quad_mms(nxt, j)
                        # bias emitted before j2's drains: the in-order V/S
                        # queue would otherwise hold it behind 4 more drains
                        # while the PE idles at the block boundary.  (Moving
                        # the whole quad + bias to j0 measures 5.3us SLOWER:
                        # the 8-MM burst stalls the PE.)
                        if j == 1:
                            quad_bias(nxt)
                if k == 0:
                    emit_out(24 * 512, HW)          # tail group
                elif k == 1:
                    emit_out(0, 4096)
                elif k == 3:
                    emit_out(4096, 8192)
                elif k == 4:
                    emit_out(8192, 10240)
            emit_out(10240, 12288)

    nc.compile()
    return nc


def _get_nc():
    if "nc" not in _CACHE:
        _CACHE["nc"] = build_nc()
    return _CACHE["nc"]


def make_in_maps(in_feats, feat_map, qry_w, qry_b, key_b, key_w):
    # 1/OUT_STEP folded into the qry projection: PSUM then holds
    # logits/OUT_STEP, so the int8 drain is a pure (rounding) copy.
    qwT = (qry_w.T / OUT_STEP).astype(np.float16)             # [256, 32]
    kwT = key_w.T.astype(np.float16)                          # [256, 32]
    bpack = np.zeros((128, 2), np.float32)
    bpack[:, 0] = np.tile(qry_b / OUT_STEP, 4)
    bpack[:, 1] = np.tile(key_b, 4)
    in_maps = []
    for c in range(N_CORES):
        b, h = divmod(c, 2)
        ifT = in_feats[b * N_PER:(b + 1) * N_PER].T.astype(np.float16)
        cpack = np.zeros((128, CPACK_W), np.float16)
        cpack[:, 0:32] = qwT[0:128]
        cpack[:, 32:64] = qwT[128:256]
        cpack[:, 64:364] = ifT[0:128]
        cpack[:, 364:664] = ifT[128:256]
        cpack[:, 664:696] = kwT[0:128]
        cpack[:, 696:728] = kwT[128:256]
        feat16 = np.ascontiguousarray(
            feat_map[b, :, h * HHALF:(h + 1) * HHALF, :]
        ).reshape(IN_DIM, HW).astype(np.float16)
        # featP: block k holds cols [2k*BLKW,(2k+2)*BLKW) = d0 cols | d1 cols
        featP = np.empty((128, FPW), np.float16)
        for k in range(N_BLK + 1):
            w = BLKW if k < N_BLK else MMN
            c0 = k * BLKW
            for d in range(2):
                featP[:, 2 * c0 + d * w:2 * c0 + (d + 1) * w] = (
                    feat16[d * 128:(d + 1) * 128, c0:c0 + w]
                )
        in_maps.append({
            "featP": featP,
            "cpack": cpack,
            "bpack": bpack,
        })
    return in_maps


def kernel(**inputs):
    in_feats = np.asarray(inputs["in_feats"], dtype=np.float32)
    feat_map = np.asarray(inputs["feat_map"], dtype=np.float32)
    qry_w = np.asarray(inputs["qry_w"], dtype=np.float32)
    qry_b = np.asarray(inputs["qry_b"], dtype=np.float32)
    key_w = np.asarray(inputs["key_w"], dtype=np.float32)
    key_b = np.asarray(inputs["key_b"], dtype=np.float32)

    from concourse import bass_utils

    nc = _get_nc()
    in_maps = make_in_maps(in_feats, feat_map, qry_w, qry_b, key_b, key_w)
    trace = os.environ.get("SEG_KERNEL_TRACE", "0") == "1"
    res = bass_utils.run_bass_kernel_spmd(
        nc, in_maps, core_ids=list(range(N_CORES)), trace=trace
    )
    _CACHE["last_result"] = res

    out = np.empty((BATCH * N_PER, FH, FW), dtype=np.float32)
    for c in range(N_CORES):
        b, h = divmod(c, 2)
        raw = res.results[c]["out"].astype(np.float32) * OUT_STEP  # [128, 3*HW]
        shard = np.empty((N_PER, HW), dtype=np.float32)
        for j, (n0, m) in enumerate(N_CHUNKS):
            shard[n0:n0 + m] = raw[0:m, j * HW:(j + 1) * HW]
        out[b * N_PER:(b + 1) * N_PER, h * HHALF:(h + 1) * HHALF, :] = (
            shard.reshape(N_PER, HHALF, FW)
        )
    return out



# revision 48
# speedup vs baseline: 1.0609x; 1.0531x over previous
"""Trainium2 Bass kernel for BaseSegHead (dynamic 1x1-conv seg logits).

Computes, for full inputs:
    qry_feats = in_feats @ qry_w.T + qry_b                  [1200, 32]
    key_map   = einsum('oc,bchw->bohw', key_w, feat_map) + key_b
    logits    = einsum('bnc,bchw->bnhw', qry_feats.reshape(4,300,32), key_map)
    out       = logits.reshape(1200, 160, 160)

Sharding: 8 cores = 4 batch images x 2 spatial (H) halves. Core c handles
batch b = c//2, rows h*80:(h+1)*80. Each core reads feat_map[b,:,rows,:],
its 300 queries, and writes a [300, 80*160] output shard -- no cross-core
communication and no duplicated feat_map reads.

Precision: matmul operands ship as fp16 (full PE rate; halves input DMA
bytes); accumulation stays fp32 in PSUM.  The OUTPUT is int8: 1/OUT_STEP
is folded into qry_w/qry_b on the host, so PSUM holds logits/OUT_STEP and
the drains quantize for free via the engines' native fp32->int8
round-to-nearest-even + saturation (verified on HW for ACT/DVE/Pool).
Host dequantizes (*OUT_STEP).  Measured error: absmax/scale 0.44%, l2
1.52% vs the 2% gate.  int8 feat_map INPUT as well measures 1.97% -- too
close to the gate; fp8 anywhere fails outright (~3-3.6%).

Layout (trace-driven, ~47-49us vs 53.1us for the fp16-output version,
60.6us original): traffic is 6.75 MB in + 3.84 MB out per core; the 16
DMA engines sustain ~26 GB/s each (~418 GB/s; per-byte rate is flat for
descriptors >=512 B, so 2-8 KB partition rows are all full-rate).  After
the int8 switch the kernel is BALANCED: input DMA ~17us, V/S drain
trains ~26us/engine, PE ~23.5us, so stall elimination, not byte count,
governs.  Input rides the SP sync HW-DGE ring (tail 512-col block first:
it primes the pipeline; then six 2048-col blocks, d0|d1 interleaved,
8 KB rows).  Out DMAs are triggered from the Pool engine via SWDGE
(~25ns seq + ~1us Pool-engine descriptor-gen each) -- a HWDGE trigger
costs ~1.2us of the issuing SEQUENCER and the SP seq was 93% busy, the
co-bottleneck of the fp16 version.  Out groups: tail first, then 4096-col
quarters with the last quarter split 2048+2048, each group's 3 chunk
triggers emitted (in Pool program order) right after the block whose
drains complete it.  PSUM drains are single-bank [*, 512] copies with
SEVEN main PSUM buffers, assigned to scalar/vector by accumulated-cost
balance with weights s=720/v=678 (copies) and s=686/v=678 (bias) -- the
BALANCE PATTERN, not the absolute ns, is what matters: re-weighting to
the measured slice times (s 576/ v 598) costs +6us.  The next block's
key-quad matmuls are split around j1 and its bias-add is emitted right
after quad half d1 (before j2's drains): in-order V/S queues otherwise
hold the bias behind 4 more drains while the PE idles ~2.3us at block
boundaries.  A dummy activation at t~0 preloads the scalar ACT table
(~1.3us ACT_TABLE_LOAD).

Known dead ends (measured on HW): PE warm-up matmuls (HAM never
un-throttles; all matmuls run at the 1.2 GHz cold rate), 2-bank paired
drains (depth 7->3 singles->pairs: +2.6us; j2-only pairs at depth 1:
+14us -- PSUM depth beats instruction count), emitting the whole next
quad + bias after j0 (8-MM burst stalls PE: +5.3us), ct/bt consts on the
ACT HWDGE queue (+2.3us), per-chunk tail triggers inside the j-loop
(+0.7us), 1024-col final-group split (neutral/worse), per-512-col final
out DMAs (+4.8us), the priming tail block via Pool SWDGE (neutral: its
desc-gen starts ~1.9us into the Pool BB and the bytes land no earlier),
per-band [32, 300] q_sb activations (engine cost is COLUMN-driven, so
4 small ACTs cost 4x one [128, 300] ACT: +2.4us), d0/d1-split input DMAs
for ramp blocks (+1.7us: the extra SP DGE configs delay later blocks).
GPSIMD cannot access PSUM (no Pool drains); only SP/ACT/Pool can trigger
DMAs; PSUM APs starting at partition != 0 are limited to one
32-partition group.  Run-to-run HW variance is ~ +/-0.9us.

TensorE array tiling: the key projection (M=32) runs 4-way column-tiled
into one PSUM bank per quad of hw-tiles; one bias-activation drains four
tiles. The main einsum (K=32) runs 4-way row-tiled: hw-tile t keeps its
q and key_map operands on SBUF partitions 32*(t%4), so consecutive tiles
issue to distinct PE row-groups and overlap on the array.
"""

import os
import sys

sys.path.insert(0, "/opt/trn_rl_repo")
os.environ.setdefault("MYCRO_LOCAL_CACHE", "1")

import numpy as np

BATCH = 4
N_PER = 300
IN_DIM = 256
KEY_DIM = 32
FH = FW = 160
HHALF = FH // 2            # 80 rows per core
HW = HHALF * FW            # 12800 spatial positions per core
N_CORES = 8

MMN = 512                  # matmul moving free size (one fp32 PSUM bank)
N_T = HW // MMN            # 25 hw-tiles
N_BLK = 6                  # six full 2048-col blocks (quads) + one 512 tail
BLKW = 4 * MMN             # 2048 feat columns per block
# out-DMA groups: tail tile first (it drains first), then 4096-col
# quarters (4KB int8 rows), with the final quarter split in two so the
# last DMA after the last drain is ~2us shorter.

N_CHUNKS = ((0, 128), (128, 128), (256, 44))   # query-row chunks (300 rows)
CPACK_W = 728              # fp16: qry_wT (64) + in_featsT (600) + key_wT (64)
FPW = 2 * HW               # featP width: d0|d1 interleaved per block

# int8 output quantization: logits are computed as logits/OUT_STEP on device
# (1/OUT_STEP folded into qry_w/qry_b on host), drained to int8 with the
# engines' native round-to-nearest-even + saturation, and rescaled on host.
# max |logit| = 24.24 -> max |q| = 125.7 < 127: no saturation. Measured
# error: absmax/scale 0.40%, l2 1.52% (gate 2%).
OUT_STEP = 24.5 / 127.0

_CACHE = {}


def build_nc():
    import concourse.bass as bass
    import concourse.bacc as bacc
    import concourse.mybir as mybir
    from concourse import tile

    f32 = mybir.dt.float32
    f16 = mybir.dt.float16
    i8 = mybir.dt.int8
    Ident = mybir.ActivationFunctionType.Identity

    nc = bacc.Bacc("TRN2", target_bir_lowering=False, debug=False)

    featP = nc.dram_tensor("featP", [128, FPW], f16, kind="ExternalInput")
    cpack = nc.dram_tensor("cpack", [128, CPACK_W], f16, kind="ExternalInput")
    bpack = nc.dram_tensor("bpack", [128, 2], f32, kind="ExternalInput")
    out = nc.dram_tensor("out", [128, 3 * HW], i8, kind="ExternalOutput")

    with tile.TileContext(nc) as tc:
        with (
            tc.tile_pool(name="const", bufs=1) as cpool,
            tc.tile_pool(name="fpool", bufs=1) as fpool,
            tc.tile_pool(name="opool", bufs=1) as opool,
            tc.tile_pool(name="kmap", bufs=1) as kpool,
            tc.tile_pool(name="ps_main", bufs=7, space=bass.MemorySpace.PSUM) as ps_main,
            tc.tile_pool(name="ps_small", bufs=1, space=bass.MemorySpace.PSUM) as ps_small,
        ):
            # --- DMA ring head: consts, then the 7 paired feat blocks ----
            # ct/bt ride the ACT sequencer's HWDGE queue: their completion
            # semaphores are then independent of the SP input firehose
            # (sharing its queues delays the completion update ~2us behind
            # the queued feat blocks, stalling the qry projection), and the
            # SP queue's first trigger becomes the priming tail block.
            ct = cpool.tile([128, CPACK_W], f16, name="ct")
            nc.sync.dma_start(ct[:], cpack[:])
            bt = cpool.tile([128, 2], f32, name="bt")
            nc.sync.dma_start(bt[:], bpack[:])
            qw = (ct[:, 0:32], ct[:, 32:64])
            inT = (ct[:, 64:364], ct[:, 364:664])
            kw = (ct[:, 664:696], ct[:, 696:728])
            qb = bt[:, 0:1]        # qry_b replicated in all four bands
            kb = bt[:, 1:2]        # key_b replicated in all four bands

            # featP block k: cols [2k*BLKW, (2k+2)*BLKW) = d0 block | d1 block
            # The tiny tail block loads FIRST: it lands ~1us before block 0
            # and primes the whole matmul->drain->out pipeline.
            fp = fpool.tile([128, FPW], f16, name="fp")
            nc.sync.dma_start(fp[:, 2 * N_BLK * BLKW:FPW],
                              featP[:, 2 * N_BLK * BLKW:FPW])
            for k in range(N_BLK):
                nc.sync.dma_start(
                    fp[:, 2 * k * BLKW:2 * (k + 1) * BLKW],
                    featP[:, 2 * k * BLKW:2 * (k + 1) * BLKW],
                )

            def feat(d, t):
                # hw-tile t, channel half d -> fp column range
                k = t // 4
                if k < N_BLK:
                    c0 = 2 * k * BLKW + d * BLKW + (t % 4) * MMN
                else:
                    c0 = 2 * N_BLK * BLKW + d * MMN
                return fp[:, c0:c0 + MMN]

            # Preload the scalar-engine activation table with a dummy op at
            # t~0 (vector memsets a scratch tile first) so the ~1.3us
            # ACT_TABLE_LOAD doesn't delay the first real activation.
            warm = cpool.tile([128, 8], f32, name="warm")
            nc.vector.memset(warm[:], 0.0)
            warm16 = cpool.tile([128, 8], f16, name="warm16")
            nc.scalar.activation(warm16[:], warm[:, 0:8], Ident, bias=warm[:, 0:1])

            # --- qry projection, 4-way column-tiled (4 band copies) -------
            # (Engine cost is COLUMN-driven: splitting the one [128, 300]
            # activation into per-band [32, 300] pieces quadruples its cost
            # and measures +2us.  PSUM APs starting at partition != 0 are
            # also limited to one 32-partition group.)
            qp = ps_small.tile([128, MMN], f32, name="qp", tag="kp")
            for b in range(4):
                for d in range(2):
                    nc.tensor.matmul(
                        qp[32 * b:32 * b + 32, 0:N_PER],
                        qw[d],
                        inT[d],
                        start=(d == 0),
                        stop=(d == 1),
                        tile_position=(0, 32 * b),
                    )
            q_sb = cpool.tile([128, N_PER], f16, name="q_sb")
            nc.scalar.activation(q_sb[:], qp[:, 0:N_PER], Ident, bias=qb)

            # --- key_map: 4-way column-tiled, banded layout ---------------
            # hw-tile t lives on SBUF partitions 32*(t%4), columns
            # (t//4)*512; one [128,512] PSUM bank holds a whole quad and is
            # drained by a single bias-activation.
            key_map = kpool.tile([128, 7 * MMN], f16, name="key_map")

            # Drains and key-quad bias-adds are assigned to scalar/vector by
            # accumulated-cost balance (GPSIMD cannot access PSUM, so Pool
            # only runs the out-DMA SWDGE triggers).
            acc = {"s": 0.0, "v": 0.0, "g": 0.0}

            def drain(dst, src, cs=720, cv=678):
                if acc["s"] + cs < acc["v"] + cv:
                    nc.scalar.copy(dst, src)
                    acc["s"] += cs
                else:
                    nc.vector.tensor_copy(dst, src)
                    acc["v"] += cv

            def bias_add(dst, src, bias):
                if acc["s"] + 686 < acc["v"] + 678:
                    nc.scalar.activation(dst, src, Ident, bias=bias)
                    acc["s"] += 686
                else:
                    nc.vector.tensor_scalar_add(dst, src, bias)
                    acc["v"] += 678

            KP = {}

            def quad_mms(k, d):
                # d outer, band inner: each round's four column-group
                # matmuls overlap on the PE array.
                if d == 0:
                    KP[k] = ps_small.tile([128, MMN], f32, name=f"kp_{k}", tag="kp")
                nb = min(4, N_T - 4 * k)
                for b in range(nb):
                    nc.tensor.matmul(
                        KP[k][32 * b:32 * b + 32, :],
                        kw[d],
                        feat(d, 4 * k + b),
                        start=(d == 0),
                        stop=(d == 1),
                        tile_position=(0, 32 * b),
                    )

            def quad_bias(k):
                nb = min(4, N_T - 4 * k)
                p = 32 * nb
                bias_add(key_map[0:p, k * MMN:(k + 1) * MMN], KP[k][0:p, :],
                         kb[0:p, :])

            def key_quad(k):
                quad_mms(k, 0)
                quad_mms(k, 1)
                quad_bias(k)

            # --- output row-buffers: one [*, 12800] int8 tile per chunk ---
            OB = [opool.tile([128, HW], i8, name=f"ob_{j}") for j in range(3)]

            # --- main einsum: 4-way row-tiled over band b = t%4 -----------
            # chunk-outer / tile-inner order: adjacent matmuls target
            # different PE row-groups and overlap on the array.  (Pairing
            # even/odd tiles into two-bank PSUM tiles with one [m, 1024]
            # drain measures 2.6us SLOWER: pipeline depth drops 7 -> 3
            # and the PE stalls on PSUM-free.)
            def main_tiles(j, tiles):
                n0, m = N_CHUNKS[j]
                for t in tiles:
                    b = t % 4
                    kcol = (t // 4) * MMN
                    mp = ps_main.tile([128, MMN], f32, name=f"mp_{t}_{n0}", tag="mp")
                    nc.tensor.matmul(
                        mp[:m, :],
                        q_sb[32 * b:32 * b + 32, n0:n0 + m],
                        key_map[32 * b:32 * b + 32, kcol:kcol + MMN],
                        tile_position=(32 * b, 0),
                    )
                    drain(OB[j][:m, t * MMN:(t + 1) * MMN], mp[:m, :])

            # out DMAs are issued from the Pool engine (SWDGE): ~25ns of
            # sequencer time per trigger vs ~1.2us of HWDGE config that
            # would serialize on the Sync sequencer (which handles the
            # input stream).  Pool is an in-order engine that also runs
            # drains now, so each group's triggers are interleaved into
            # the emission stream right after the block that completes
            # the group.
            def emit_out_j(j, c0, c1):
                m = N_CHUNKS[j][1]
                acc["g"] += 1040
                nc.gpsimd.dma_start(
                    out[0:m, j * HW + c0:j * HW + c1], OB[j][0:m, c0:c1]
                )

            def emit_out(c0, c1):
                for j in range(3):
                    emit_out_j(j, c0, c1)

            # Interleave: tail quad/tile first (its data lands first), then
            # each key quad feeds its four hw-tiles.  Quads are emitted one
            # block AHEAD of their main tiles so the tensor stream never
            # stalls waiting for the current block's bias-add: while
            # bias_add(k) pends, the PE runs quad k+1's matmuls; the next
            # quad's matmuls are spread between the current block's
            # chunk-mains (4-MM half-rounds, not one 8-MM burst).
            key_quad(6)
            for j in range(3):
                main_tiles(j, (24,))
            quad_mms(0, 0)
            quad_mms(0, 1)
            quad_bias(0)
            for k in range(6):
                nxt = k + 1 if k < 5 else None
                for j in range(3):
                    main_tiles(j, range(4 * k, 4 * k + 4))
                    if nxt is not None and j < 2:
                        quad_mms(nxt, j)
                        # bias emitted before j2's drains: the in-order V/S
                        # queue would otherwise hold it behind 4 more drains
                        # while the PE idles at the block boundary.  (Moving
                        # the whole quad + bias to j0 measures 5.3us SLOWER:
                        # the 8-MM burst stalls the PE.)
                        if j == 1:
                            quad_bias(nxt)
                if k == 0:
                    emit_out(24 * 512, HW)          # tail group
                elif k == 1:
                    emit_out(0, 4096)
                elif k == 3:
                    emit_out(4096, 8192)
                elif k == 4:
                    emit_out(8192, 10240)
            emit_out(10240, 12288)

    nc.compile()
    return nc


def _get_nc():
    if "nc" not in _CACHE:
        _CACHE["nc"] = build_nc()
    return _CACHE["nc"]


def make_in_maps(in_feats, feat_map, qry_w, qry_b, key_b, key_w):
    # 1/OUT_STEP folded into the qry projection: PSUM then holds
    # logits/OUT_STEP, so the int8 drain is a pure (rounding) copy.
    qwT = (qry_w.T / OUT_STEP).astype(np.float16)             # [256, 32]
    kwT = key_w.T.astype(np.float16)                          # [256, 32]
    bpack = np.zeros((128, 2), np.float32)
    bpack[:, 0] = np.tile(qry_b / OUT_STEP, 4)
    bpack[:, 1] = np.tile(key_b, 4)
    in_maps = []
    for c in range(N_CORES):
        b, h = divmod(c, 2)
        ifT = in_feats[b * N_PER:(b + 1) * N_PER].T.astype(np.float16)
        cpack = np.zeros((128, CPACK_W), np.float16)
        cpack[:, 0:32] = qwT[0:128]
        cpack[:, 32:64] = qwT[128:256]
        cpack[:, 64:364] = ifT[0:128]
        cpack[:, 364:664] = ifT[128:256]
        cpack[:, 664:696] = kwT[0:128]
        cpack[:, 696:728] = kwT[128:256]
        feat16 = np.ascontiguousarray(
            feat_map[b, :, h * HHALF:(h + 1) * HHALF, :]
        ).reshape(IN_DIM, HW).astype(np.float16)
        # featP: block k holds cols [2k*BLKW,(2k+2)*BLKW) = d0 cols | d1 cols
        featP = np.empty((128, FPW), np.float16)
        for k in range(N_BLK + 1):
            w = BLKW if k < N_BLK else MMN
            c0 = k * BLKW
            for d in range(2):
                featP[:, 2 * c0 + d * w:2 * c0 + (d + 1) * w] = (
                    feat16[d * 128:(d + 1) * 128, c0:c0 + w]
                )
        in_maps.append({
            "featP": featP,
            "cpack": cpack,
            "bpack": bpack,
        })
    return in_maps


def kernel(**inputs):
    in_feats = np.asarray(inputs["in_feats"], dtype=np.float32)
    feat_map = np.asarray(inputs["feat_map"], dtype=np.float32)
    qry_w = np.asarray(inputs["qry_w"], dtype=np.float32)
    qry_b = np.asarray(inputs["qry_b"], dtype=np.float32)
    key_w = np.asarray(inputs["key_w"], dtype=np.float32)
    key_b = np.asarray(inputs["key_b"], dtype=np.float32)

    from concourse import bass_utils

    nc = _get_nc()
    in_maps = make_in_maps(in_feats, feat_map, qry_w, qry_b, key_b, key_w)
    trace = os.environ.get("SEG_KERNEL_TRACE", "0") == "1"
    res = bass_utils.run_bass_kernel_spmd(
        nc, in_maps, core_ids=list(range(N_CORES)), trace=trace
    )
    _CACHE["last_result"] = res

    out = np.empty((BATCH * N_PER, FH, FW), dtype=np.float32)
    for c in range(N_CORES):
        b, h = divmod(c, 2)
        raw = res.results[c]["out"].astype(np.float32) * OUT_STEP  # [128, 3*HW]
        shard = np.empty((N_PER, HW), dtype=np.float32)
        for j, (n0, m) in enumerate(N_CHUNKS):
            shard[n0:n0 + m] = raw[0:m, j * HW:(j + 1) * HW]
        out[b * N_PER:(b + 1) * N_PER, h * HHALF:(h + 1) * HHALF, :] = (
            shard.reshape(N_PER, HHALF, FW)
        )
    return out

